# revision 1
# baseline (speedup 1.0000x reference)
"""Trainium2 Bass kernel for nn_DirectionalScan (2D directional diagonal-SSM + projection).

Math: for each of two directions (scan over h, scan over w),
    y[t] = sum_n Cm*Bm * sum_{u<=t} A^(t-u) x[u]  + D_skip*x[t]
then out = (y_h + y_v) @ Wp.T + b_proj.

Device decomposition (validated in fp64/fp32 numpy): chunked SSM with chunk Q=16,
all heavy work on the PE as matmuls:
  - intra-chunk causal Toeplitz (kernel K[d,tau]=sum_n CB*A^tau, + D on the diagonal)
  - chunk-boundary states via a per-chunk increment matmul + a tiny 4-step recurrence
  - inter-chunk contribution via a CB*A^(i+1) matmul accumulated into the same PSUM
  - fused output projection with Wp.T

Sharding: 8 cores; core k handles batch b=k//2 and half=k%2:
  vertical  (scan over w): sequences (b, h in [32*half, 32*half+32))
  horizontal(scan over h): sequences (b, w in [32*half, 32*half+32))
Each core projects its two partial y tensors separately (projection is linear);
the host scatter-adds the two 2048-token contributions into the full output.
"""
import os
from contextlib import ExitStack

import numpy as np

import concourse.bass as bass
import concourse.bacc as bacc
import concourse.tile as tile
from concourse import mybir
from concourse.bass_utils import run_bass_kernel_spmd
from concourse.masks import make_identity

F32 = mybir.dt.float32
F16 = mybir.dt.float16
NP_CDT = np.float16
ALU = mybir.AluOpType
B, H, W, D, N = 4, 64, 64, 512, 8
L, Q, C, SEQ = 64, 16, 4, 32   # seq len, chunk size, n chunks, seqs/core/direction
NOCT = 64                      # octets of 8 channels
NG = 32                        # 2-octet groups


# ----------------------------------------------------------------------------
# host-side weight packing
# ----------------------------------------------------------------------------

def _precompute_weights(A, Bm, Cm, D_skip, Wp):
    A64, B64, C64 = A.astype(np.float64), Bm.astype(np.float64), Cm.astype(np.float64)
    CB = C64 * B64                                   # [D, N]
    Apow = np.stack([A64 ** t for t in range(Q + 1)])  # [Q+1, D, N]
    Kconv = np.einsum("dn,tdn->dt", CB, Apow)        # [D, Q+1]
    T = np.zeros((D, Q, Q))
    for i in range(Q):
        for j in range(i + 1):
            T[:, i, j] = Kconv[:, i - j]
    T += np.eye(Q)[None] * D_skip.astype(np.float64)[:, None, None]

    # K-rows ordered (j16, d8): row = j*8 + d8 (matches the PE-transpose output)
    W_T = np.zeros((NOCT, 128, 128))
    W_P = np.zeros((NOCT, 128, 64))
    for o in range(NOCT):
        for d8 in range(8):
            d = o * 8 + d8
            for j in range(Q):
                W_T[o, j * 8 + d8, d8::8] = T[d, :, j]
                W_P[o, j * 8 + d8, d8 * 8:d8 * 8 + 8] = Apow[Q - 1 - j, d]
    W_CBA = np.zeros((NG, 128, 256))
    for g in range(NG):
        for o2 in range(2):
            for d8 in range(8):
                d = g * 16 + o2 * 8 + d8
                for n in range(N):
                    row = o2 * 64 + d8 * 8 + n
                    W_CBA[g, row, o2 * 128 + d8:o2 * 128 + 128:8] = (
                        CB[d, n] * Apow[1:Q + 1, d, n]
                    )
    A16 = np.zeros((128, NG))
    for g in range(NG):
        for o2 in range(2):
            for d8 in range(8):
                d = g * 16 + o2 * 8 + d8
                A16[o2 * 64 + d8 * 8:o2 * 64 + d8 * 8 + 8, g] = Apow[Q, d]
    A16 = np.repeat(A16, SEQ, axis=1)  # [128, (g32, s32)]
    WPT = np.ascontiguousarray(Wp.astype(np.float64).T.reshape(4, 128, 512))
    return (W_T.astype(NP_CDT), W_P.astype(NP_CDT), W_CBA.astype(NP_CDT),
            A16.astype(np.float32), WPT.astype(NP_CDT))


# ----------------------------------------------------------------------------
# device program
# ----------------------------------------------------------------------------

def _one_direction(tc, pools, consts, x_view, z_view, tag, batched_out):
    """x_view: DRAM AP [c4, seq32, j16, d512]; z_view: [c4, seq32, i16, e512]."""
    nc = tc.nc
    (xnat_pool, xperm_pool, xt_pool, s_pool, y_pool, yt_pool, out_pool,
     psA, psyw, psG, psout) = pools
    w_t_sb, w_p_sb, w_cba_sb, a16_sb, wpt_sb, ident = consts

    x_nat = xnat_pool.tile([128, Q * D], F16, tag="xnat", name=f"xnat_{tag}")
    for c in range(C):
        nc.sync.dma_start(
            x_nat[c * SEQ:(c + 1) * SEQ, :].rearrange("s (j d) -> s j d", j=Q),
            x_view[c])
    # reorder (j16, d512) -> (o64, j16, d8): each octet one contiguous 128 block
    x_perm = xperm_pool.tile([128, Q * D], F16, tag="xperm", name=f"xperm_{tag}")
    x_src = x_nat[:].rearrange("p (j o e) -> p o j e", j=Q, o=NOCT, e=8)
    x_dst = x_perm[:].rearrange("p (o j e) -> p o j e", j=Q, o=NOCT, e=8)
    nc.vector.tensor_copy(x_dst[:, 0:32], x_src[:, 0:32])
    nc.vector.tensor_copy(x_dst[:, 32:64], x_src[:, 32:64])

    y_sb = y_pool.tile([128, NOCT * 128], F16, tag="y", name=f"y_{tag}")

    # phase T: transpose all octets into SBUF xt tiles (4 octets per tile)
    xts = []
    for og in range(16):
        ps_t = psA.tile([128, 512], F16, tag="ps_t")
        xt = xt_pool.tile([128, 512], F16, tag="xt")
        for oo in range(4):
            o = og * 4 + oo
            nc.tensor.transpose(
                ps_t[:, oo * 128:(oo + 1) * 128],
                x_perm[:, o * 128:(o + 1) * 128], ident)
        if og % 2 == 0:
            nc.scalar.copy(xt[:], ps_t[:])
        else:
            nc.vector.tensor_copy(xt[:], ps_t[:])
        xts.append(xt)

    def xt_oct(o):
        return xts[o // 4][:, (o % 4) * 128:(o % 4) * 128 + 128]

    # phase G: chunk-increment matmuls, 4 groups (8 octets) per PSUM bank,
    # then the batched 4-group chunk-state recurrence on DVE
    s_tiles = []
    for q in range(8):  # og-pairs: 8 octets each
        ps_g = psG.tile([128, 512], F32, tag="ps_g")
        for k in range(8):
            o = q * 8 + k
            half = (o % 2) * 64
            col = (k // 2) * 128
            nc.tensor.matmul(
                ps_g[half:half + 64, col:col + 128],
                w_p_sb[:, o * 64:o * 64 + 64], xt_oct(o),
                start=True, stop=True, skip_group_check=True,
                tile_position=(0, half))
        s4 = s_pool.tile([128, 512], F16, tag="s")
        sv = s4[:].rearrange("p (g c s) -> p g c s", g=4, c=C, s=SEQ)
        gv = ps_g[:].rearrange("p (g c s) -> p g c s", g=4, c=C, s=SEQ)
        a16b = a16_sb[:, q * 128:q * 128 + 128].rearrange(
            "p (g s) -> p g s", g=4)
        nc.gpsimd.memset(sv[:, :, 0, :], 0.0)
        nc.vector.tensor_copy(sv[:, :, 1, :], gv[:, :, 0, :])
        for cc in (2, 3):
            nc.vector.tensor_mul(sv[:, :, cc, :], sv[:, :, cc - 1, :], a16b)
            nc.vector.tensor_add(sv[:, :, cc, :], sv[:, :, cc, :], gv[:, :, cc - 1, :])
        s_tiles.append(s4)

    # phase B: intra-chunk matmuls + inter-chunk accumulation; 2 groups per bank
    for og in range(16):
        ps_yw = psyw.tile([128, 512], F32, tag="ps_yw")
        for oo in range(4):
            o = og * 4 + oo
            nc.tensor.matmul(ps_yw[:, oo * 128:oo * 128 + 128], xt_oct(o),
                             w_t_sb[:, o * 128:o * 128 + 128],
                             start=(oo == 0), stop=False, skip_group_check=True)
        for gg in range(2):
            g = og * 2 + gg
            s4 = s_tiles[g // 4]
            nc.tensor.matmul(ps_yw[:, gg * 256:gg * 256 + 256],
                             s4[:, (g % 4) * 128:(g % 4) * 128 + 128],
                             w_cba_sb[:, g * 256:g * 256 + 256],
                             start=False, stop=(gg == 1), skip_group_check=True)
        # scatter into y_sb layout (i16, o64, d8); ps_yw cols are (o4, i16, d8)
        y_dst = y_sb[:].rearrange("p (i og o e) -> p i og o e",
                                  i=Q, og=16, o=4, e=8)[:, :, og]
        ps_src = ps_yw[:].rearrange("p (o i e) -> p i o e", o=4, i=Q, e=8)
        if og % 2 == 0:
            nc.vector.tensor_copy(y_dst, ps_src)
        else:
            nc.scalar.copy(y_dst, ps_src)

    # projection: per i transpose y slice to [d, sc] then matmul with WpT
    for iq in range(4):
        out_sb = out_pool.tile([128, 4 * 512], F32, tag="osb")
        for ii in range(4):
            i = iq * 4 + ii
            ps_yt = psA.tile([128, 512], F16, tag="ps_t")
            for dc in range(4):
                nc.tensor.transpose(
                    ps_yt[:, dc * 128:dc * 128 + 128],
                    y_sb[:, i * 512 + dc * 128:i * 512 + (dc + 1) * 128], ident)
            yt = yt_pool.tile([128, 512], F16, tag="yt")
            if i % 2 == 0:
                nc.scalar.copy(yt[:], ps_yt[:])
            else:
                nc.vector.tensor_copy(yt[:], ps_yt[:])
            ps_o = psout.tile([128, 512], F32, tag="ps_o")
            for dc in range(4):
                nc.tensor.matmul(ps_o[:], yt[:, dc * 128:dc * 128 + 128],
                                 wpt_sb[:, dc * 512:dc * 512 + 512],
                                 start=(dc == 0), stop=(dc == 3))
            if i % 2 == 0:
                nc.vector.tensor_copy(out_sb[:, ii * 512:ii * 512 + 512], ps_o[:])
            else:
                nc.scalar.copy(out_sb[:, ii * 512:ii * 512 + 512], ps_o[:])
        if batched_out:
            nc.gpsimd.dma_start(z_view[:, :, iq * 4:iq * 4 + 4, :], out_sb[:])
        else:
            for ii in range(4):
                nc.gpsimd.dma_start(z_view[:, :, iq * 4 + ii, :],
                                  out_sb[:, ii * 512:ii * 512 + 512])


def _kernel_body(ctx, tc, aps):
    nc = tc.nc
    const_pool = ctx.enter_context(tc.tile_pool(name="consts", bufs=1))
    xnat_pool = ctx.enter_context(tc.tile_pool(name="xnat", bufs=2))
    xperm_pool = ctx.enter_context(tc.tile_pool(name="xperm", bufs=2))
    xt_pool = ctx.enter_context(tc.tile_pool(name="xt", bufs=20))
    s_pool = ctx.enter_context(tc.tile_pool(name="s", bufs=8))
    y_pool = ctx.enter_context(tc.tile_pool(name="y", bufs=2))
    yt_pool = ctx.enter_context(tc.tile_pool(name="yt", bufs=2))
    out_pool = ctx.enter_context(tc.tile_pool(name="osb", bufs=3))
    psA = ctx.enter_context(tc.tile_pool(name="psA", bufs=2, space="PSUM"))
    psyw = ctx.enter_context(tc.tile_pool(name="psyw", bufs=2, space="PSUM"))
    psG = ctx.enter_context(tc.tile_pool(name="psG", bufs=2, space="PSUM"))
    psout = ctx.enter_context(tc.tile_pool(name="psout", bufs=2, space="PSUM"))
    pools = (xnat_pool, xperm_pool, xt_pool, s_pool, y_pool, yt_pool, out_pool,
             psA, psyw, psG, psout)

    w_t_sb = const_pool.tile([128, NOCT * 128], F16, name="w_t_sb")
    w_p_sb = const_pool.tile([128, NOCT * 64], F16, name="w_p_sb")
    w_cba_sb = const_pool.tile([128, NG * 256], F16, name="w_cba_sb")
    a16_sb = const_pool.tile([128, NG * SEQ], F32, name="a16_sb")
    wpt_sb = const_pool.tile([128, 4 * 512], F16, name="wpt_sb")
    ident = const_pool.tile([128, 128], F16, name="ident")
    nc.scalar.dma_start(w_t_sb[:].rearrange("p (o m) -> p o m", o=NOCT),
                        aps["w_t"].rearrange("o p m -> p o m"))
    nc.scalar.dma_start(w_p_sb[:].rearrange("p (o m) -> p o m", o=NOCT),
                        aps["w_p"].rearrange("o p m -> p o m"))
    nc.scalar.dma_start(w_cba_sb[:].rearrange("p (g m) -> p g m", g=NG),
                        aps["w_cba"].rearrange("g p m -> p g m"))
    nc.scalar.dma_start(a16_sb[:], aps["a16"])
    nc.scalar.dma_start(wpt_sb[:].rearrange("p (c m) -> p c m", c=4),
                        aps["wpt"].rearrange("c p m -> p c m"))
    make_identity(nc, ident[:])
    consts = (w_t_sb[:], w_p_sb[:], w_cba_sb[:], a16_sb[:], wpt_sb[:], ident[:])

    # vertical: xv [32 (h-seq), 64 (w=pos), 512] ; zv same indexing
    xv_view = aps["xv"].rearrange("s (c j) d -> c s j d", c=C, j=Q)
    zv_view = aps["zv"].rearrange("s (c i) d -> c s i d", c=C, i=Q)
    _one_direction(tc, pools, consts, xv_view, zv_view, "v", True)
    # horizontal: xh [64 (h=pos), 32 (w-seq), 512]
    xh_view = aps["xh"].rearrange("(c j) s d -> c s j d", c=C, j=Q)
    zh_view = aps["zh"].rearrange("(c i) s d -> c s i d", c=C, i=Q)
    _one_direction(tc, pools, consts, xh_view, zh_view, "h", False)


def build_program(n_cores=8):
    nc = bacc.Bacc("TRN2", target_bir_lowering=False, debug=False,
                   enable_asserts=False, num_devices=n_cores)
    aps = {
        "xv": nc.dram_tensor("xv", [SEQ, L, D], F16, kind="ExternalInput").ap(),
        "xh": nc.dram_tensor("xh", [L, SEQ, D], F16, kind="ExternalInput").ap(),
        "w_t": nc.dram_tensor("w_t", [NOCT, 128, 128], F16, kind="ExternalInput").ap(),
        "w_p": nc.dram_tensor("w_p", [NOCT, 128, 64], F16, kind="ExternalInput").ap(),
        "w_cba": nc.dram_tensor("w_cba", [NG, 128, 256], F16, kind="ExternalInput").ap(),
        "a16": nc.dram_tensor("a16", [128, NG * SEQ], F32, kind="ExternalInput").ap(),
        "wpt": nc.dram_tensor("wpt", [4, 128, 512], F16, kind="ExternalInput").ap(),
        "zv": nc.dram_tensor("zv", [SEQ, L, D], F32, kind="ExternalOutput").ap(),
        "zh": nc.dram_tensor("zh", [L, SEQ, D], F32, kind="ExternalOutput").ap(),
    }
    with tile.TileContext(nc) as tc:
        with ExitStack() as ctx:
            _kernel_body(ctx, tc, aps)
    nc.compile()
    return nc


_PROGRAM = None


def _get_program():
    global _PROGRAM
    if _PROGRAM is None:
        _PROGRAM = build_program()
    return _PROGRAM


def make_in_maps(x, A, Bm, Cm, D_skip, Wp):
    W_T, W_P, W_CBA, A16, WPT = _precompute_weights(A, Bm, Cm, D_skip, Wp)
    xg = np.ascontiguousarray(x, dtype=np.float32).reshape(B, H, W, D)
    in_maps = []
    for k in range(8):
        b, half = k // 2, k % 2
        in_maps.append({
            "xv": np.ascontiguousarray(xg[b, 32 * half:32 * half + 32]).astype(NP_CDT),
            "xh": np.ascontiguousarray(xg[b, :, 32 * half:32 * half + 32]).astype(NP_CDT),
            "w_t": W_T, "w_p": W_P, "w_cba": W_CBA, "a16": A16, "wpt": WPT,
        })
    return in_maps


def assemble_output(results, b_proj):
    out = np.zeros((B, H, W, D), np.float32)
    for k in range(8):
        b, half = k // 2, k % 2
        out[b, 32 * half:32 * half + 32, :, :] += results[k]["zv"]
        out[b, :, 32 * half:32 * half + 32, :] += results[k]["zh"]
    out += np.asarray(b_proj, dtype=np.float32)
    return out.reshape(B, H * W, D)


def kernel(x, h, w, A, Bm, Cm, D_skip, Wp, b_proj, **_kw):
    nc = _get_program()
    in_maps = make_in_maps(np.asarray(x), np.asarray(A), np.asarray(Bm),
                           np.asarray(Cm), np.asarray(D_skip), np.asarray(Wp))
    res = run_bass_kernel_spmd(nc, in_maps, list(range(8)))
    return assemble_output(res.results, np.asarray(b_proj))



# revision 6
# speedup vs baseline: 1.6465x; 1.6465x over previous
"""Trainium2 Bass kernel for nn_DirectionalScan — fused-projection variant.

Sharding: core k = (b = k//2, half = k%2) owns tokens (b, all h, w in half).
  horizontal (scan over h): 32 own w-columns, 64 h positions -> self-contained.
  vertical   (scan over w): 64 h-rows, scan over the core's own 32 w positions
    (2 chunks); the scan state entering the half is supplied by the host as
    s_pre (the sharding halo: zeros for half=0, the w<32 prefix state for
    half=1, a 0.7%-of-flops boundary summary).
Both directions' y are transposed to [d, token] tiles in a SHARED token order,
summed there, and projected ONCE per core (2048 tokens) with Wp.T.

Token order in yt/z: t = ih*128 + ch*32 + w_loc  (h = ch*16+ih, w = half*32+w_loc).
"""
from contextlib import ExitStack

import numpy as np

import concourse.bass as bass
import concourse.bacc as bacc
import concourse.tile as tile
from concourse import mybir
from concourse.bass_utils import run_bass_kernel_spmd
from concourse.masks import make_identity

F32 = mybir.dt.float32
F16 = mybir.dt.float16
NP_CDT = np.float16
B, H, W, D, N = 4, 64, 64, 512, 8
Q, NOCT, NG = 16, 64, 32
SV, CV = 64, 4     # vertical: 64 h-row seqs, 4 w-chunks (own = {2,3})
SH, CH = 32, 4     # horizontal: 32 w-col seqs, 4 h-chunks (all own)
VPERM = (np.arange(64) % 4) * 16 + np.arange(64) // 4  # s' -> h row


# ----------------------------------------------------------------------------
# host-side packing
# ----------------------------------------------------------------------------

def _precompute_weights(A, Bm, Cm, D_skip, Wp):
    A64, B64, C64 = A.astype(np.float64), Bm.astype(np.float64), Cm.astype(np.float64)
    CB = C64 * B64
    Apow = np.stack([A64 ** t for t in range(Q + 1)])
    Kconv = np.einsum("dn,tdn->dt", CB, Apow)
    T = np.zeros((D, Q, Q))
    for i in range(Q):
        for j in range(i + 1):
            T[:, i, j] = Kconv[:, i - j]
    T += np.eye(Q)[None] * D_skip.astype(np.float64)[:, None, None]

    W_T = np.zeros((NOCT, 128, 128))
    W_P = np.zeros((NOCT, 128, 64))
    for o in range(NOCT):
        for d8 in range(8):
            d = o * 8 + d8
            for j in range(Q):
                W_T[o, j * 8 + d8, d8::8] = T[d, :, j]
                W_P[o, j * 8 + d8, d8 * 8:d8 * 8 + 8] = Apow[Q - 1 - j, d]
    W_CBA = np.zeros((NG, 128, 256))
    A16 = np.zeros((128, NG))
    for g in range(NG):
        for o2 in range(2):
            for d8 in range(8):
                d = g * 16 + o2 * 8 + d8
                for n in range(N):
                    row = o2 * 64 + d8 * 8 + n
                    W_CBA[g, row, o2 * 128 + d8:o2 * 128 + 128:8] = (
                        CB[d, n] * Apow[1:Q + 1, d, n]
                    )
                A16[o2 * 64 + d8 * 8:o2 * 64 + d8 * 8 + 8, g] = Apow[Q, d]
    w_t = np.ascontiguousarray(W_T.transpose(1, 0, 2)).reshape(128, NOCT * 128)
    w_p = np.ascontiguousarray(W_P.transpose(1, 0, 2)).reshape(128, NOCT * 64)
    w_cba = np.ascontiguousarray(W_CBA.transpose(1, 0, 2)).reshape(128, NG * 256)
    wpt = np.ascontiguousarray(
        Wp.astype(np.float64).T.reshape(4, 128, 512).transpose(1, 0, 2)
    ).reshape(128, 4 * 512)
    # recurrence multipliers, broadcast over seqs: [128, (g32, s)]
    a16v = np.repeat(A16, SV, axis=1)   # [128, 2048]
    a16h = np.repeat(A16, SH, axis=1)   # [128, 1024]
    return (w_t.astype(NP_CDT), w_p.astype(NP_CDT), w_cba.astype(NP_CDT),
            wpt.astype(NP_CDT), a16v.astype(NP_CDT), a16h.astype(NP_CDT))


def _pack_xt(xs, S, C):
    """xs: [S seqs, C*16 positions, D] -> [(j16,d8)=128, (o64, cC, sS)] f16."""
    a = xs.reshape(S, C, Q, NOCT, 8)             # s, c, j, o, d8
    a = a.transpose(2, 4, 3, 1, 0)               # j, d8, o, c, s
    return np.ascontiguousarray(a).reshape(128, NOCT * C * S).astype(NP_CDT)


# ----------------------------------------------------------------------------
# device program
# ----------------------------------------------------------------------------

def _kernel_body(ctx, tc, aps):
    nc = tc.nc
    const_pool = ctx.enter_context(tc.tile_pool(name="consts", bufs=1))
    xtv_pool = ctx.enter_context(tc.tile_pool(name="xtv", bufs=1))
    xth_pool = ctx.enter_context(tc.tile_pool(name="xth", bufs=1))
    g_pool = ctx.enter_context(tc.tile_pool(name="gall", bufs=1))
    y_pool = ctx.enter_context(tc.tile_pool(name="y", bufs=2))
    yt_pool = ctx.enter_context(tc.tile_pool(name="yt", bufs=1))
    out_pool = ctx.enter_context(tc.tile_pool(name="osb", bufs=3))
    psG = ctx.enter_context(tc.tile_pool(name="psG", bufs=2, space="PSUM"))
    psyw = ctx.enter_context(tc.tile_pool(name="psyw", bufs=2, space="PSUM"))
    psA = ctx.enter_context(tc.tile_pool(name="psA", bufs=2, space="PSUM"))
    psout = ctx.enter_context(tc.tile_pool(name="psout", bufs=2, space="PSUM"))

    # ---- input DMAs: 1MB pieces, first-use-first, spread over both HWDGE ----
    xtv = [xtv_pool.tile([128, 4096], F16, name=f"xt_v{t}") for t in range(2)]
    xth = [xth_pool.tile([128, 4096], F16, name=f"xt_h{t}") for t in range(2)]
    w_p_sb = const_pool.tile([128, NOCT * 64], F16, name="w_p_sb")
    wt = [const_pool.tile([128, 4096], F16, name=f"w_t_sb{t}") for t in range(2)]
    wc = [const_pool.tile([128, 4096], F16, name=f"w_cba_sb{t}") for t in range(2)]
    wpt_sb = const_pool.tile([128, 4 * 512], F16, name="wpt_sb")
    a16v_sb = const_pool.tile([128, NG * SV], F16, name="a16v_sb")
    a16h_sb = const_pool.tile([128, NG * SH], F16, name="a16h_sb")
    spre_sb = const_pool.tile([128, NG * SV], F16, name="spre_sb")
    ident = const_pool.tile([128, 128], F16, name="ident")
    xv_ap, xh_ap = aps["xv"], aps["xh"]
    nc.sync.dma_start(w_p_sb[:], aps["w_p"])
    nc.scalar.dma_start(xtv[1][:], xv_ap[:, 4096:])
    nc.sync.dma_start(xtv[0][:], xv_ap[:, :4096])
    nc.scalar.dma_start(spre_sb[:], aps["s_pre"])
    nc.scalar.dma_start(a16v_sb[:], aps["a16v"])
    nc.scalar.dma_start(a16h_sb[:], aps["a16h"])
    nc.sync.dma_start(xth[0][:], xh_ap[:, :4096])
    nc.scalar.dma_start(wc[0][:], aps["w_cba"][:, :4096])
    nc.sync.dma_start(wt[0][:], aps["w_t"][:, :4096])
    nc.scalar.dma_start(xth[1][:], xh_ap[:, 4096:])
    nc.sync.dma_start(wt[1][:], aps["w_t"][:, 4096:])
    nc.scalar.dma_start(wc[1][:], aps["w_cba"][:, 4096:])
    nc.sync.dma_start(wpt_sb[:], aps["wpt"])
    make_identity(nc, ident[:])

    def xt_oct_v(o):   # per octet: own (c2, s64) = 128 cols
        return xtv[o // 32][:, (o % 32) * 128:(o % 32) * 128 + 128]

    xt_v_hi = xt_oct_v  # intra reads the same own-half columns

    def xt_oct_h(o):
        return xth[o // 32][:, (o % 32) * 128:(o % 32) * 128 + 128]

    def w_t_oct(o):
        return wt[o // 32][:, (o % 32) * 128:(o % 32) * 128 + 128]

    def w_cba_g(g):
        return wc[g // 16][:, (g % 16) * 256:(g % 16) * 256 + 256]

    def w_p_oct(o):
        return w_p_sb[:, o * 64:o * 64 + 64]

    def cp(k, dst, src):
        if k % 2 == 0:
            nc.scalar.copy(dst, src)
        else:
            nc.vector.tensor_copy(dst, src)

    # ---- phase G, both directions ----
    gall_v = g_pool.tile([128, 8 * 512], F16, name="gall_v")   # (q8,g4,c2,s64)
    sall_v = g_pool.tile([128, 8 * 512], F16, name="sall_v")   # (q8,g4,c2,s64)
    gall_h = g_pool.tile([128, 8 * 512], F16, name="gall_h")   # (q8,g4,c4,s32)
    sall_h = g_pool.tile([128, 8 * 512], F16, name="sall_h")   # (q8,g4,c4,s32)

    for q in range(8):   # vertical: 8 octets per PSUM bank
        ps_g = psG.tile([128, 512], F32, tag="ps_g")
        for k in range(8):
            o = q * 8 + k
            half = (o % 2) * 64
            col = (k // 2) * 128
            nc.tensor.matmul(ps_g[half:half + 64, col:col + 128],
                             w_p_oct(o), xt_oct_v(o),
                             start=True, stop=True, skip_group_check=True,
                             tile_position=(0, half))
        cp(q, gall_v[:, q * 512:(q + 1) * 512], ps_g[:])
    for q in range(8):   # horizontal: 8 octets per PSUM bank
        ps_g = psG.tile([128, 512], F32, tag="ps_g")
        for k in range(8):
            o = q * 8 + k
            half = (o % 2) * 64
            col = (k // 2) * 128
            nc.tensor.matmul(ps_g[half:half + 64, col:col + 128],
                             w_p_oct(o), xt_oct_h(o),
                             start=True, stop=True, skip_group_check=True,
                             tile_position=(0, half))
        cp(q + 1, gall_h[:, q * 512:(q + 1) * 512], ps_g[:])

    # ---- chunk-state recurrences (gpsimd, SBUF only) ----
    gv = gall_v[:].rearrange("p (q g c s) -> p q g c s", q=8, g=4, c=2, s=SV)
    sv = sall_v[:].rearrange("p (q g c s) -> p q g c s", q=8, g=4, c=2, s=SV)
    av = a16v_sb[:].rearrange("p (q g s) -> p q g s", q=8, g=4, s=SV)
    spre = spre_sb[:].rearrange("p (q g s) -> p q g s", q=8, g=4, s=SV)
    for i, (lo, hi) in enumerate(((0, 4), (4, 8))):
        eng = nc.vector if i == 0 else nc.gpsimd
        svq, gvq, avq = sv[:, lo:hi], gv[:, lo:hi], av[:, lo:hi]
        eng.tensor_copy(svq[:, :, :, 0], spre[:, lo:hi])
        eng.tensor_mul(svq[:, :, :, 1], spre[:, lo:hi], avq)
        eng.tensor_add(svq[:, :, :, 1], svq[:, :, :, 1], gvq[:, :, :, 0])
    gh = gall_h[:].rearrange("p (q g c s) -> p q g c s", q=8, g=4, c=4, s=SH)
    sh = sall_h[:].rearrange("p (q g c s) -> p q g c s", q=8, g=4, c=4, s=SH)
    ah = a16h_sb[:].rearrange("p (q g s) -> p q g s", q=8, g=4, s=SH)
    nc.gpsimd.memset(sh[:, :, :, 0], 0.0)
    for i, (lo, hi) in enumerate(((0, 4), (4, 8))):
        eng = nc.vector if i == 0 else nc.gpsimd
        shq, ghq, ahq = sh[:, lo:hi], gh[:, lo:hi], ah[:, lo:hi]
        nc.scalar.copy(shq[:, :, :, 1], ghq[:, :, :, 0])
        eng.tensor_mul(shq[:, :, :, 2], shq[:, :, :, 1], ahq)
        eng.tensor_add(shq[:, :, :, 2], shq[:, :, :, 2], ghq[:, :, :, 1])
        eng.tensor_mul(shq[:, :, :, 3], shq[:, :, :, 2], ahq)
        eng.tensor_add(shq[:, :, :, 3], shq[:, :, :, 3], ghq[:, :, :, 2])

    # ---- phase B, both directions ----
    y_v = y_pool.tile([128, NOCT * 128], F16, tag="y", name="y_v")
    y_h = y_pool.tile([128, NOCT * 128], F16, tag="y", name="y_h")
    for og in range(16):  # vertical, own chunks {2,3} only
        ps_yw = psyw.tile([128, 512], F32, tag="ps_yw")
        for oo in range(4):
            o = og * 4 + oo
            nc.tensor.matmul(ps_yw[:, oo * 128:oo * 128 + 128],
                             xt_v_hi(o), w_t_oct(o),
                             start=(oo == 0), stop=False, skip_group_check=True)
        for gg in range(2):
            g = og * 2 + gg
            nc.tensor.matmul(ps_yw[:, gg * 256:gg * 256 + 256],
                             sall_v[:, g * 128:g * 128 + 128], w_cba_g(g),
                             start=False, stop=(gg == 1), skip_group_check=True)
        y_dst = y_v[:].rearrange("p (i og o e) -> p i og o e",
                                 i=Q, og=16, o=4, e=8)[:, :, og]
        ps_src = ps_yw[:].rearrange("p (o i e) -> p i o e", o=4, i=Q, e=8)
        cp(og, y_dst, ps_src)
    for og in range(16):  # horizontal, all 4 chunks
        ps_yw = psyw.tile([128, 512], F32, tag="ps_yw")
        for oo in range(4):
            o = og * 4 + oo
            nc.tensor.matmul(ps_yw[:, oo * 128:oo * 128 + 128], xt_oct_h(o),
                             w_t_oct(o),
                             start=(oo == 0), stop=False, skip_group_check=True)
        for gg in range(2):
            g = og * 2 + gg
            nc.tensor.matmul(ps_yw[:, gg * 256:gg * 256 + 256],
                             sall_h[:, g * 128:g * 128 + 128], w_cba_g(g),
                             start=False, stop=(gg == 1), skip_group_check=True)
        y_dst = y_h[:].rearrange("p (i og o e) -> p i og o e",
                                 i=Q, og=16, o=4, e=8)[:, :, og]
        ps_src = ps_yw[:].rearrange("p (o i e) -> p i o e", o=4, i=Q, e=8)
        cp(og + 1, y_dst, ps_src)

    # ---- transpose to [d, t] and merge; t = iw*128 + cv*64 + s',
    # where the host packs vertical seqs in order s' = ih*4 + ch  ----
    yt = yt_pool.tile([128, 4 * 2048], F16, name="yt")  # (dc4, t2048)
    # vertical: y_v partitions (cv2, s'64) == t order within block iw
    for iw in range(16):
        ps_yt = psA.tile([128, 512], F16, tag="ps_t")
        for dc in range(4):
            nc.tensor.transpose(
                ps_yt[:, dc * 128:(dc + 1) * 128],
                y_v[:, iw * 512 + dc * 128:iw * 512 + (dc + 1) * 128], ident)
        # contiguous: yt[:, (dc, iw-block)] <- ps_yt[:, (dc, m)]
        dst = yt[:].rearrange("p (dc b m) -> p b dc m", dc=4, b=16)[:, iw]
        nc.scalar.copy(dst, ps_yt[:].rearrange("p (dc m) -> p dc m", dc=4))
    # horizontal: y_h partitions (ch4, w32); accumulate with run-4 writes
    for ih in range(16):
        ps_yt = psA.tile([128, 512], F16, tag="ps_t")
        for dc in range(4):
            nc.tensor.transpose(
                ps_yt[:, dc * 128:(dc + 1) * 128],
                y_h[:, ih * 512 + dc * 128:ih * 512 + (dc + 1) * 128], ident)
        # t = iw*128 + cv*64 + ih*4 + ch ; ps_yt cols (dc4, ch4, cv2, iw16)
        for cv in range(2):
            dst = yt[:].rearrange("p (dc iw cv u) -> p cv dc iw u",
                                  dc=4, iw=16, cv=2, u=64)[
                :, cv, :, :, ih * 4:ih * 4 + 4]
            srcv = ps_yt[:].rearrange("p (dc ch cv iw) -> p cv dc iw ch",
                                      dc=4, ch=4, cv=2, iw=16)[:, cv]
            nc.vector.tensor_add(dst, dst, srcv)

    # ---- single fused projection over 2048 tokens ----
    for blk in range(16):
        ps_o = psout.tile([128, 512], F32, tag="ps_o")
        for dc in range(4):
            nc.tensor.matmul(ps_o[:],
                             yt[:, dc * 2048 + blk * 128:dc * 2048 + blk * 128 + 128],
                             wpt_sb[:, dc * 512:dc * 512 + 512],
                             start=(dc == 0), stop=(dc == 3))
        out_sb = out_pool.tile([128, 512], F16, tag="osb")
        cp(blk, out_sb[:], ps_o[:])
        nc.sync.dma_start(aps["z"][blk * 128:(blk + 1) * 128, :], out_sb[:])


def build_program(n_cores=8):
    nc = bacc.Bacc("TRN2", target_bir_lowering=False, debug=False,
                   enable_asserts=False, num_devices=n_cores)
    aps = {
        "xv": nc.dram_tensor("xv", [128, NOCT * 128], F16, kind="ExternalInput").ap(),
        "xh": nc.dram_tensor("xh", [128, NOCT * 128], F16, kind="ExternalInput").ap(),
        "w_t": nc.dram_tensor("w_t", [128, NOCT * 128], F16, kind="ExternalInput").ap(),
        "w_p": nc.dram_tensor("w_p", [128, NOCT * 64], F16, kind="ExternalInput").ap(),
        "w_cba": nc.dram_tensor("w_cba", [128, NG * 256], F16,
                                kind="ExternalInput").ap(),
        "wpt": nc.dram_tensor("wpt", [128, 4 * 512], F16, kind="ExternalInput").ap(),
        "a16v": nc.dram_tensor("a16v", [128, NG * SV], F16, kind="ExternalInput").ap(),
        "a16h": nc.dram_tensor("a16h", [128, NG * SH], F16, kind="ExternalInput").ap(),
        "s_pre": nc.dram_tensor("s_pre", [128, NG * SV], F16,
                                kind="ExternalInput").ap(),
        "z": nc.dram_tensor("z", [2048, D], F16, kind="ExternalOutput").ap(),
    }
    with tile.TileContext(nc) as tc:
        with ExitStack() as ctx:
            _kernel_body(ctx, tc, aps)
    nc.compile()
    return nc


_PROGRAM = None


def _get_program():
    global _PROGRAM
    if _PROGRAM is None:
        _PROGRAM = build_program()
    return _PROGRAM


def make_in_maps(x, A, Bm, Cm, D_skip, Wp):
    w_t, w_p, w_cba, wpt, a16v, a16h = _precompute_weights(A, Bm, Cm, D_skip, Wp)
    xg = np.ascontiguousarray(x, dtype=np.float32).reshape(B, H, W, D)
    A64 = A.astype(np.float64)
    # B-normalized boundary state after w=31 (B is folded into W_CBA)
    P = A64[None] ** np.arange(31, -1, -1, dtype=np.float64)[:, None, None]
    s_bnd = np.einsum("udn,bsud->bsdn", P,
                      xg[:, VPERM, :32, :].astype(np.float64))
    # pack [b, s(h), d, n] -> [(o2,d8,n)=128, (q8, g4, s64)]
    sp = s_bnd.reshape(B, SV, NG, 2, 8, N)       # b, s, g32, o2, d8, n
    sp = sp.transpose(0, 3, 4, 5, 2, 1)          # b, o2, d8, n, g, s
    sp = sp.reshape(B, 128, NG * SV).astype(NP_CDT)
    in_maps = []
    for k in range(8):
        b, half = k // 2, k % 2
        # vertical: 64 h-row seqs in order s' = (h%16)*4 + h//16
        xs_v = np.ascontiguousarray(xg[b, VPERM, 32 * half:32 * half + 32])
        # horizontal: 32 own w-col seqs; positions = h 0..63
        xs_h = np.ascontiguousarray(
            xg[b, :, 32 * half:32 * half + 32].transpose(1, 0, 2))
        s_pre = sp[b] if half == 1 else np.zeros_like(sp[b])
        # (sp already packed with the permuted seq order)
        in_maps.append({
            "xv": _pack_xt(xs_v, SV, 2), "xh": _pack_xt(xs_h, SH, 4),
            "w_t": w_t, "w_p": w_p, "w_cba": w_cba, "wpt": wpt,
            "a16v": a16v, "a16h": a16h, "s_pre": s_pre,
        })
    return in_maps


def assemble_output(results, b_proj):
    out = np.zeros((B, H, W, D), np.float32)
    for k in range(8):
        b, half = k // 2, k % 2
        z = results[k]["z"].astype(np.float32)      # [2048,512], t=(iw,cv,ih,ch)
        zr = z.reshape(16, 2, 16, 4, D).transpose(3, 2, 1, 0, 4)  # ch,ih,cv,iw
        out[b, :, 32 * half:32 * half + 32] = zr.reshape(H, SH, D)
    out += np.asarray(b_proj, dtype=np.float32)
    return out.reshape(B, H * W, D)


def kernel(x, h, w, A, Bm, Cm, D_skip, Wp, b_proj, **_kw):
    nc = _get_program()
    in_maps = make_in_maps(np.asarray(x), np.asarray(A), np.asarray(Bm),
                           np.asarray(Cm), np.asarray(D_skip), np.asarray(Wp))
    res = run_bass_kernel_spmd(nc, in_maps, list(range(8)))
    return assemble_output(res.results, np.asarray(b_proj))


# revision 7
# speedup vs baseline: 1.6515x; 1.0030x over previous
"""Trainium2 Bass kernel for nn_DirectionalScan — fused-projection variant.

Sharding: core k = (b = k//2, half = k%2) owns tokens (b, all h, w in half).
  horizontal (scan over h): 32 own w-columns, 64 h positions -> self-contained.
  vertical   (scan over w): 64 h-rows, scan over the core's own 32 w positions
    (2 chunks); the scan state entering the half is supplied by the host as
    s_pre (the sharding halo: zeros for half=0, the w<32 prefix state for
    half=1, a 0.7%-of-flops boundary summary).
Both directions' y are transposed to [d, token] tiles in a SHARED token order,
summed there, and projected ONCE per core (2048 tokens) with Wp.T.

Token order in yt/z: t = iw*128 + cv*64 + ih*4 + ch  (h = ch*16+ih,
w_loc = cv*16+iw); vertical seqs are host-packed in order s' = ih*4+ch.
"""
from contextlib import ExitStack

import numpy as np

import concourse.bass as bass
import concourse.bacc as bacc
import concourse.tile as tile
from concourse import mybir
from concourse.bass_utils import run_bass_kernel_spmd
from concourse.masks import make_identity

F32 = mybir.dt.float32
F16 = mybir.dt.float16
NP_CDT = np.float16
B, H, W, D, N = 4, 64, 64, 512, 8
Q, NOCT, NG = 16, 64, 32
SV, CV = 64, 4     # vertical: 64 h-row seqs, 4 w-chunks (own = {2,3})
SH, CH = 32, 4     # horizontal: 32 w-col seqs, 4 h-chunks (all own)
VPERM = (np.arange(64) % 4) * 16 + np.arange(64) // 4  # s' -> h row


# ----------------------------------------------------------------------------
# host-side packing
# ----------------------------------------------------------------------------

def _precompute_weights(A, Bm, Cm, D_skip, Wp):
    A64, B64, C64 = A.astype(np.float64), Bm.astype(np.float64), Cm.astype(np.float64)
    CB = C64 * B64
    Apow = np.stack([A64 ** t for t in range(Q + 1)])
    Kconv = np.einsum("dn,tdn->dt", CB, Apow)
    T = np.zeros((D, Q, Q))
    for i in range(Q):
        for j in range(i + 1):
            T[:, i, j] = Kconv[:, i - j]
    T += np.eye(Q)[None] * D_skip.astype(np.float64)[:, None, None]

    W_T = np.zeros((NOCT, 128, 128))
    W_P = np.zeros((NOCT, 128, 64))
    for o in range(NOCT):
        for d8 in range(8):
            d = o * 8 + d8
            for j in range(Q):
                W_T[o, j * 8 + d8, d8::8] = T[d, :, j]
                W_P[o, j * 8 + d8, d8 * 8:d8 * 8 + 8] = Apow[Q - 1 - j, d]
    W_CBA = np.zeros((NG, 128, 256))
    A16 = np.zeros((128, NG))
    for g in range(NG):
        for o2 in range(2):
            for d8 in range(8):
                d = g * 16 + o2 * 8 + d8
                for n in range(N):
                    row = o2 * 64 + d8 * 8 + n
                    W_CBA[g, row, o2 * 128 + d8:o2 * 128 + 128:8] = (
                        CB[d, n] * Apow[1:Q + 1, d, n]
                    )
                A16[o2 * 64 + d8 * 8:o2 * 64 + d8 * 8 + 8, g] = Apow[Q, d]
    w_t = np.ascontiguousarray(W_T.transpose(1, 0, 2)).reshape(128, NOCT * 128)
    w_p = np.ascontiguousarray(W_P.transpose(1, 0, 2)).reshape(128, NOCT * 64)
    w_cba = np.ascontiguousarray(W_CBA.transpose(1, 0, 2)).reshape(128, NG * 256)
    wpt = np.ascontiguousarray(
        Wp.astype(np.float64).T.reshape(4, 128, 512).transpose(1, 0, 2)
    ).reshape(128, 4 * 512)
    # recurrence multipliers, broadcast over seqs: [128, (g32, s)]
    a16v = np.repeat(A16, SV, axis=1)   # [128, 2048]
    a16h = np.repeat(A16, SH, axis=1)   # [128, 1024]
    return (w_t.astype(NP_CDT), w_p.astype(NP_CDT), w_cba.astype(NP_CDT),
            wpt.astype(NP_CDT), a16v.astype(NP_CDT), a16h.astype(NP_CDT))


def _pack_xt(xs, S, C):
    """xs: [S seqs, C*16 positions, D] -> [(j16,d8)=128, (o64, cC, sS)] f16."""
    a = xs.reshape(S, C, Q, NOCT, 8)             # s, c, j, o, d8
    a = a.transpose(2, 4, 3, 1, 0)               # j, d8, o, c, s
    return np.ascontiguousarray(a).reshape(128, NOCT * C * S).astype(NP_CDT)


# ----------------------------------------------------------------------------
# device program
# ----------------------------------------------------------------------------

def _kernel_body(ctx, tc, aps):
    nc = tc.nc
    const_pool = ctx.enter_context(tc.tile_pool(name="consts", bufs=1))
    xtv_pool = ctx.enter_context(tc.tile_pool(name="xtv", bufs=1))
    xth_pool = ctx.enter_context(tc.tile_pool(name="xth", bufs=1))
    g_pool = ctx.enter_context(tc.tile_pool(name="gall", bufs=1))
    y_pool = ctx.enter_context(tc.tile_pool(name="y", bufs=2))
    yt_pool = ctx.enter_context(tc.tile_pool(name="yt", bufs=1))
    out_pool = ctx.enter_context(tc.tile_pool(name="osb", bufs=3))
    psG = ctx.enter_context(tc.tile_pool(name="psG", bufs=2, space="PSUM"))
    psyw = ctx.enter_context(tc.tile_pool(name="psyw", bufs=2, space="PSUM"))
    psA = ctx.enter_context(tc.tile_pool(name="psA", bufs=2, space="PSUM"))
    psout = ctx.enter_context(tc.tile_pool(name="psout", bufs=2, space="PSUM"))

    # ---- input DMAs: 1MB pieces, first-use-first, spread over both HWDGE ----
    xtv = [xtv_pool.tile([128, 4096], F16, name=f"xt_v{t}") for t in range(2)]
    xth = [xth_pool.tile([128, 4096], F16, name=f"xt_h{t}") for t in range(2)]
    w_p_sb = const_pool.tile([128, NOCT * 64], F16, name="w_p_sb")
    wt = [const_pool.tile([128, 4096], F16, name=f"w_t_sb{t}") for t in range(2)]
    wc = [const_pool.tile([128, 4096], F16, name=f"w_cba_sb{t}") for t in range(2)]
    wpt_sb = const_pool.tile([128, 4 * 512], F16, name="wpt_sb")
    a16v_sb = const_pool.tile([128, NG * SV], F16, name="a16v_sb")
    a16h_sb = const_pool.tile([128, NG * SH], F16, name="a16h_sb")
    spre_sb = const_pool.tile([128, NG * SV], F16, name="spre_sb")
    ident = const_pool.tile([128, 128], F16, name="ident")
    xv_ap, xh_ap = aps["xv"], aps["xh"]
    nc.scalar.dma_start(w_p_sb[:], aps["w_p"])
    nc.sync.dma_start(xtv[0][:], xv_ap[:, :4096])
    nc.scalar.dma_start(xtv[1][:], xv_ap[:, 4096:])
    nc.scalar.dma_start(spre_sb[:], aps["s_pre"])
    nc.scalar.dma_start(a16v_sb[:], aps["a16v"])
    nc.scalar.dma_start(a16h_sb[:], aps["a16h"])
    nc.sync.dma_start(xth[0][:], xh_ap[:, :4096])
    nc.scalar.dma_start(wc[0][:], aps["w_cba"][:, :4096])
    nc.sync.dma_start(wt[0][:], aps["w_t"][:, :4096])
    nc.scalar.dma_start(xth[1][:], xh_ap[:, 4096:])
    nc.sync.dma_start(wt[1][:], aps["w_t"][:, 4096:])
    nc.scalar.dma_start(wc[1][:], aps["w_cba"][:, 4096:])
    nc.sync.dma_start(wpt_sb[:], aps["wpt"])
    make_identity(nc, ident[:])

    def xt_oct_v(o):   # per octet: own (c2, s64) = 128 cols
        return xtv[o // 32][:, (o % 32) * 128:(o % 32) * 128 + 128]

    xt_v_hi = xt_oct_v  # intra reads the same own-half columns

    def xt_oct_h(o):
        return xth[o // 32][:, (o % 32) * 128:(o % 32) * 128 + 128]

    def w_t_oct(o):
        return wt[o // 32][:, (o % 32) * 128:(o % 32) * 128 + 128]

    def w_cba_g(g):
        return wc[g // 16][:, (g % 16) * 256:(g % 16) * 256 + 256]

    def w_p_oct(o):
        return w_p_sb[:, o * 64:o * 64 + 64]

    def cp(k, dst, src):
        if k % 2 == 0:
            nc.scalar.copy(dst, src)
        else:
            nc.vector.tensor_copy(dst, src)

    # ---- phase G, both directions ----
    gall_v = g_pool.tile([128, 8 * 512], F16, name="gall_v")   # (q8,g4,c2,s64)
    sall_v = g_pool.tile([128, 8 * 512], F16, name="sall_v")   # (q8,g4,c2,s64)
    gall_h = g_pool.tile([128, 8 * 512], F16, name="gall_h")   # (q8,g4,c4,s32)
    sall_h = g_pool.tile([128, 8 * 512], F16, name="sall_h")   # (q8,g4,c4,s32)

    for q in range(8):   # vertical: 8 octets per PSUM bank
        ps_g = psG.tile([128, 512], F32, tag="ps_g")
        for k in range(8):
            o = q * 8 + k
            half = (o % 2) * 64
            col = (k // 2) * 128
            nc.tensor.matmul(ps_g[half:half + 64, col:col + 128],
                             w_p_oct(o), xt_oct_v(o),
                             start=True, stop=True, skip_group_check=True,
                             tile_position=(0, half))
        cp(q, gall_v[:, q * 512:(q + 1) * 512], ps_g[:])
    for q in range(8):   # horizontal: 8 octets per PSUM bank
        ps_g = psG.tile([128, 512], F32, tag="ps_g")
        for k in range(8):
            o = q * 8 + k
            half = (o % 2) * 64
            col = (k // 2) * 128
            nc.tensor.matmul(ps_g[half:half + 64, col:col + 128],
                             w_p_oct(o), xt_oct_h(o),
                             start=True, stop=True, skip_group_check=True,
                             tile_position=(0, half))
        cp(q + 1, gall_h[:, q * 512:(q + 1) * 512], ps_g[:])

    # ---- chunk-state recurrences (gpsimd, SBUF only) ----
    gv = gall_v[:].rearrange("p (q g c s) -> p q g c s", q=8, g=4, c=2, s=SV)
    sv = sall_v[:].rearrange("p (q g c s) -> p q g c s", q=8, g=4, c=2, s=SV)
    av = a16v_sb[:].rearrange("p (q g s) -> p q g s", q=8, g=4, s=SV)
    spre = spre_sb[:].rearrange("p (q g s) -> p q g s", q=8, g=4, s=SV)
    for i, (lo, hi) in enumerate(((0, 4), (4, 8))):
        eng = nc.vector if i == 0 else nc.gpsimd
        svq, gvq, avq = sv[:, lo:hi], gv[:, lo:hi], av[:, lo:hi]
        eng.tensor_copy(svq[:, :, :, 0], spre[:, lo:hi])
        eng.tensor_mul(svq[:, :, :, 1], spre[:, lo:hi], avq)
        eng.tensor_add(svq[:, :, :, 1], svq[:, :, :, 1], gvq[:, :, :, 0])
    gh = gall_h[:].rearrange("p (q g c s) -> p q g c s", q=8, g=4, c=4, s=SH)
    sh = sall_h[:].rearrange("p (q g c s) -> p q g c s", q=8, g=4, c=4, s=SH)
    ah = a16h_sb[:].rearrange("p (q g s) -> p q g s", q=8, g=4, s=SH)
    nc.gpsimd.memset(sh[:, :, :, 0], 0.0)
    for i, (lo, hi) in enumerate(((0, 4), (4, 8))):
        eng = nc.vector if i == 0 else nc.gpsimd
        shq, ghq, ahq = sh[:, lo:hi], gh[:, lo:hi], ah[:, lo:hi]
        nc.scalar.copy(shq[:, :, :, 1], ghq[:, :, :, 0])
        eng.tensor_mul(shq[:, :, :, 2], shq[:, :, :, 1], ahq)
        eng.tensor_add(shq[:, :, :, 2], shq[:, :, :, 2], ghq[:, :, :, 1])
        eng.tensor_mul(shq[:, :, :, 3], shq[:, :, :, 2], ahq)
        eng.tensor_add(shq[:, :, :, 3], shq[:, :, :, 3], ghq[:, :, :, 2])

    # ---- phase B, both directions ----
    y_v = y_pool.tile([128, NOCT * 128], F16, tag="y", name="y_v")
    y_h = y_pool.tile([128, NOCT * 128], F16, tag="y", name="y_h")
    for og in range(16):  # vertical, own chunks {2,3} only
        ps_yw = psyw.tile([128, 512], F32, tag="ps_yw")
        for oo in range(4):
            o = og * 4 + oo
            nc.tensor.matmul(ps_yw[:, oo * 128:oo * 128 + 128],
                             xt_v_hi(o), w_t_oct(o),
                             start=(oo == 0), stop=False, skip_group_check=True)
        for gg in range(2):
            g = og * 2 + gg
            nc.tensor.matmul(ps_yw[:, gg * 256:gg * 256 + 256],
                             sall_v[:, g * 128:g * 128 + 128], w_cba_g(g),
                             start=False, stop=(gg == 1), skip_group_check=True)
        y_dst = y_v[:].rearrange("p (i og o e) -> p i og o e",
                                 i=Q, og=16, o=4, e=8)[:, :, og]
        ps_src = ps_yw[:].rearrange("p (o i e) -> p i o e", o=4, i=Q, e=8)
        cp(og, y_dst, ps_src)
    for og in range(16):  # horizontal, all 4 chunks
        ps_yw = psyw.tile([128, 512], F32, tag="ps_yw")
        for oo in range(4):
            o = og * 4 + oo
            nc.tensor.matmul(ps_yw[:, oo * 128:oo * 128 + 128], xt_oct_h(o),
                             w_t_oct(o),
                             start=(oo == 0), stop=False, skip_group_check=True)
        for gg in range(2):
            g = og * 2 + gg
            nc.tensor.matmul(ps_yw[:, gg * 256:gg * 256 + 256],
                             sall_h[:, g * 128:g * 128 + 128], w_cba_g(g),
                             start=False, stop=(gg == 1), skip_group_check=True)
        y_dst = y_h[:].rearrange("p (i og o e) -> p i og o e",
                                 i=Q, og=16, o=4, e=8)[:, :, og]
        ps_src = ps_yw[:].rearrange("p (o i e) -> p i o e", o=4, i=Q, e=8)
        cp(og + 1, y_dst, ps_src)

    # ---- transpose to [d, t] and merge; t = iw*128 + cv*64 + s',
    # where the host packs vertical seqs in order s' = ih*4 + ch  ----
    yt = yt_pool.tile([128, 4 * 2048], F16, name="yt")  # (dc4, t2048)
    # vertical: y_v partitions (cv2, s'64) == t order within block iw
    for iw in range(16):
        ps_yt = psA.tile([128, 512], F16, tag="ps_t")
        for dc in range(4):
            nc.tensor.transpose(
                ps_yt[:, dc * 128:(dc + 1) * 128],
                y_v[:, iw * 512 + dc * 128:iw * 512 + (dc + 1) * 128], ident)
        # contiguous: yt[:, (dc, iw-block)] <- ps_yt[:, (dc, m)]
        dst = yt[:].rearrange("p (dc b m) -> p b dc m", dc=4, b=16)[:, iw]
        nc.scalar.copy(dst, ps_yt[:].rearrange("p (dc m) -> p dc m", dc=4))
    # horizontal: y_h partitions (ch4, w32); accumulate with run-4 writes
    for ih in range(16):
        ps_yt = psA.tile([128, 512], F16, tag="ps_t")
        for dc in range(4):
            nc.tensor.transpose(
                ps_yt[:, dc * 128:(dc + 1) * 128],
                y_h[:, ih * 512 + dc * 128:ih * 512 + (dc + 1) * 128], ident)
        # t = iw*128 + cv*64 + ih*4 + ch ; ps_yt cols (dc4, ch4, cv2, iw16)
        for cv in range(2):
            dst = yt[:].rearrange("p (dc iw cv u) -> p cv dc iw u",
                                  dc=4, iw=16, cv=2, u=64)[
                :, cv, :, :, ih * 4:ih * 4 + 4]
            srcv = ps_yt[:].rearrange("p (dc ch cv iw) -> p cv dc iw ch",
                                      dc=4, ch=4, cv=2, iw=16)[:, cv]
            nc.vector.tensor_add(dst, dst, srcv)

    # ---- single fused projection over 2048 tokens ----
    for blk in range(16):
        ps_o = psout.tile([128, 512], F32, tag="ps_o")
        for dc in range(4):
            nc.tensor.matmul(ps_o[:],
                             yt[:, dc * 2048 + blk * 128:dc * 2048 + blk * 128 + 128],
                             wpt_sb[:, dc * 512:dc * 512 + 512],
                             start=(dc == 0), stop=(dc == 3))
        out_sb = out_pool.tile([128, 512], F16, tag="osb")
        cp(blk, out_sb[:], ps_o[:])
        nc.sync.dma_start(aps["z"][blk * 128:(blk + 1) * 128, :], out_sb[:])


def build_program(n_cores=8):
    nc = bacc.Bacc("TRN2", target_bir_lowering=False, debug=False,
                   enable_asserts=False, num_devices=n_cores)
    aps = {
        "xv": nc.dram_tensor("xv", [128, NOCT * 128], F16, kind="ExternalInput").ap(),
        "xh": nc.dram_tensor("xh", [128, NOCT * 128], F16, kind="ExternalInput").ap(),
        "w_t": nc.dram_tensor("w_t", [128, NOCT * 128], F16, kind="ExternalInput").ap(),
        "w_p": nc.dram_tensor("w_p", [128, NOCT * 64], F16, kind="ExternalInput").ap(),
        "w_cba": nc.dram_tensor("w_cba", [128, NG * 256], F16,
                                kind="ExternalInput").ap(),
        "wpt": nc.dram_tensor("wpt", [128, 4 * 512], F16, kind="ExternalInput").ap(),
        "a16v": nc.dram_tensor("a16v", [128, NG * SV], F16, kind="ExternalInput").ap(),
        "a16h": nc.dram_tensor("a16h", [128, NG * SH], F16, kind="ExternalInput").ap(),
        "s_pre": nc.dram_tensor("s_pre", [128, NG * SV], F16,
                                kind="ExternalInput").ap(),
        "z": nc.dram_tensor("z", [2048, D], F16, kind="ExternalOutput").ap(),
    }
    with tile.TileContext(nc) as tc:
        with ExitStack() as ctx:
            _kernel_body(ctx, tc, aps)
    nc.compile()
    return nc


_PROGRAM = None


def _get_program():
    global _PROGRAM
    if _PROGRAM is None:
        _PROGRAM = build_program()
    return _PROGRAM


def make_in_maps(x, A, Bm, Cm, D_skip, Wp):
    w_t, w_p, w_cba, wpt, a16v, a16h = _precompute_weights(A, Bm, Cm, D_skip, Wp)
    xg = np.ascontiguousarray(x, dtype=np.float32).reshape(B, H, W, D)
    A64 = A.astype(np.float64)
    # B-normalized boundary state after w=31 (B is folded into W_CBA)
    P = A64[None] ** np.arange(31, -1, -1, dtype=np.float64)[:, None, None]
    s_bnd = np.einsum("udn,bsud->bsdn", P,
                      xg[:, VPERM, :32, :].astype(np.float64))
    # pack [b, s(h), d, n] -> [(o2,d8,n)=128, (q8, g4, s64)]
    sp = s_bnd.reshape(B, SV, NG, 2, 8, N)       # b, s, g32, o2, d8, n
    sp = sp.transpose(0, 3, 4, 5, 2, 1)          # b, o2, d8, n, g, s
    sp = sp.reshape(B, 128, NG * SV).astype(NP_CDT)
    in_maps = []
    for k in range(8):
        b, half = k // 2, k % 2
        # vertical: 64 h-row seqs in order s' = (h%16)*4 + h//16
        xs_v = np.ascontiguousarray(xg[b, VPERM, 32 * half:32 * half + 32])
        # horizontal: 32 own w-col seqs; positions = h 0..63
        xs_h = np.ascontiguousarray(
            xg[b, :, 32 * half:32 * half + 32].transpose(1, 0, 2))
        s_pre = sp[b] if half == 1 else np.zeros_like(sp[b])
        # (sp already packed with the permuted seq order)
        in_maps.append({
            "xv": _pack_xt(xs_v, SV, 2), "xh": _pack_xt(xs_h, SH, 4),
            "w_t": w_t, "w_p": w_p, "w_cba": w_cba, "wpt": wpt,
            "a16v": a16v, "a16h": a16h, "s_pre": s_pre,
        })
    return in_maps


def assemble_output(results, b_proj):
    out = np.zeros((B, H, W, D), np.float32)
    for k in range(8):
        b, half = k // 2, k % 2
        z = results[k]["z"].astype(np.float32)      # [2048,512], t=(iw,cv,ih,ch)
        zr = z.reshape(16, 2, 16, 4, D).transpose(3, 2, 1, 0, 4)  # ch,ih,cv,iw
        out[b, :, 32 * half:32 * half + 32] = zr.reshape(H, SH, D)
    out += np.asarray(b_proj, dtype=np.float32)
    return out.reshape(B, H * W, D)


def kernel(x, h, w, A, Bm, Cm, D_skip, Wp, b_proj, **_kw):
    nc = _get_program()
    in_maps = make_in_maps(np.asarray(x), np.asarray(A), np.asarray(Bm),
                           np.asarray(Cm), np.asarray(D_skip), np.asarray(Wp))
    res = run_bass_kernel_spmd(nc, in_maps, list(range(8)))
    return assemble_output(res.results, np.asarray(b_proj))


# revision 8
# speedup vs baseline: 1.6868x; 1.0214x over previous
"""Trainium2 Bass kernel for nn_DirectionalScan — fused-projection variant.

Sharding: core k = (b = k//2, half = k%2) owns tokens (b, all h, w in half).
  horizontal (scan over h): 32 own w-columns, 64 h positions -> self-contained.
  vertical   (scan over w): 64 h-rows, scan over the core's own 32 w positions
    (2 chunks); the scan state entering the half is supplied by the host as
    s_pre (the sharding halo: zeros for half=0, the w<32 prefix state for
    half=1, a 0.7%-of-flops boundary summary).
Both directions' y are transposed to [d, token] tiles in a SHARED token order,
summed there, and projected ONCE per core (2048 tokens) with Wp.T.

Token order in yt/z: t = iw*128 + cv*64 + ih*4 + ch  (h = ch*16+ih,
w_loc = cv*16+iw); vertical seqs are host-packed in order s' = ih*4+ch.
"""
from contextlib import ExitStack

import numpy as np

import concourse.bass as bass
import concourse.bacc as bacc
import concourse.tile as tile
from concourse import mybir
from concourse.bass_utils import run_bass_kernel_spmd
from concourse.masks import make_identity

F32 = mybir.dt.float32
F16 = mybir.dt.float16
NP_CDT = np.float16
B, H, W, D, N = 4, 64, 64, 512, 8
Q, NOCT, NG = 16, 64, 32
SV, CV = 64, 4     # vertical: 64 h-row seqs, 4 w-chunks (own = {2,3})
SH, CH = 32, 4     # horizontal: 32 w-col seqs, 4 h-chunks (all own)
VPERM = (np.arange(64) % 4) * 16 + np.arange(64) // 4  # s' -> h row


# ----------------------------------------------------------------------------
# host-side packing
# ----------------------------------------------------------------------------

def _precompute_weights(A, Bm, Cm, D_skip, Wp):
    A64, B64, C64 = A.astype(np.float64), Bm.astype(np.float64), Cm.astype(np.float64)
    CB = C64 * B64
    Apow = np.stack([A64 ** t for t in range(Q + 1)])
    Kconv = np.einsum("dn,tdn->dt", CB, Apow)
    T = np.zeros((D, Q, Q))
    for i in range(Q):
        for j in range(i + 1):
            T[:, i, j] = Kconv[:, i - j]
    T += np.eye(Q)[None] * D_skip.astype(np.float64)[:, None, None]

    W_T = np.zeros((NOCT, 128, 128))
    W_P = np.zeros((NOCT, 128, 64))
    for o in range(NOCT):
        for d8 in range(8):
            d = o * 8 + d8
            for j in range(Q):
                W_T[o, j * 8 + d8, d8::8] = T[d, :, j]
                W_P[o, j * 8 + d8, d8 * 8:d8 * 8 + 8] = Apow[Q - 1 - j, d]
    W_CBA = np.zeros((NG, 128, 256))
    A16 = np.zeros((128, NG))
    for g in range(NG):
        for o2 in range(2):
            for d8 in range(8):
                d = g * 16 + o2 * 8 + d8
                for n in range(N):
                    row = o2 * 64 + d8 * 8 + n
                    W_CBA[g, row, o2 * 128 + d8:o2 * 128 + 128:8] = (
                        CB[d, n] * Apow[1:Q + 1, d, n]
                    )
                A16[o2 * 64 + d8 * 8:o2 * 64 + d8 * 8 + 8, g] = Apow[Q, d]
    w_t = np.ascontiguousarray(W_T.transpose(1, 0, 2)).reshape(128, NOCT * 128)
    w_p = np.ascontiguousarray(W_P.transpose(1, 0, 2)).reshape(128, NOCT * 64)
    w_cba = np.ascontiguousarray(W_CBA.transpose(1, 0, 2)).reshape(128, NG * 256)
    wpt = np.ascontiguousarray(
        Wp.astype(np.float64).T.reshape(4, 128, 512).transpose(1, 0, 2)
    ).reshape(128, 4 * 512)
    # recurrence multipliers, broadcast over seqs: [128, (g32, s)]
    a16v = np.repeat(A16, SV, axis=1)   # [128, 2048]
    a16h = np.repeat(A16, SH, axis=1)   # [128, 1024]
    return (w_t.astype(NP_CDT), w_p.astype(NP_CDT), w_cba.astype(NP_CDT),
            wpt.astype(NP_CDT), a16v.astype(NP_CDT), a16h.astype(NP_CDT))


def _pack_xt(xs, S, C):
    """xs: [S seqs, C*16 positions, D] -> [(j16,d8)=128, (o64, cC, sS)] f16."""
    a = xs.reshape(S, C, Q, NOCT, 8)             # s, c, j, o, d8
    a = a.transpose(2, 4, 3, 1, 0)               # j, d8, o, c, s
    return np.ascontiguousarray(a).reshape(128, NOCT * C * S).astype(NP_CDT)


# ----------------------------------------------------------------------------
# device program
# ----------------------------------------------------------------------------

def _kernel_body(ctx, tc, aps):
    nc = tc.nc
    const_pool = ctx.enter_context(tc.tile_pool(name="consts", bufs=1))
    xtv_pool = ctx.enter_context(tc.tile_pool(name="xtv", bufs=1))
    xth_pool = ctx.enter_context(tc.tile_pool(name="xth", bufs=1))
    g_pool = ctx.enter_context(tc.tile_pool(name="gall", bufs=1))
    y_pool = ctx.enter_context(tc.tile_pool(name="y", bufs=2))
    yt_pool = ctx.enter_context(tc.tile_pool(name="yt", bufs=1))
    out_pool = ctx.enter_context(tc.tile_pool(name="osb", bufs=3))
    psG = ctx.enter_context(tc.tile_pool(name="psG", bufs=2, space="PSUM"))
    psyw = ctx.enter_context(tc.tile_pool(name="psyw", bufs=2, space="PSUM"))
    psA = ctx.enter_context(tc.tile_pool(name="psA", bufs=2, space="PSUM"))
    psout = ctx.enter_context(tc.tile_pool(name="psout", bufs=2, space="PSUM"))

    # ---- input DMAs: 1MB pieces, first-use-first, spread over both HWDGE ----
    xtv = [xtv_pool.tile([128, 4096], F16, name=f"xt_v{t}") for t in range(2)]
    xth = [xth_pool.tile([128, 4096], F16, name=f"xt_h{t}") for t in range(2)]
    w_p_sb = const_pool.tile([128, NOCT * 64], F16, name="w_p_sb")
    wt = [const_pool.tile([128, 4096], F16, name=f"w_t_sb{t}") for t in range(2)]
    wc = [const_pool.tile([128, 4096], F16, name=f"w_cba_sb{t}") for t in range(2)]
    wpt_sb = const_pool.tile([128, 4 * 512], F16, name="wpt_sb")
    a16v_sb = const_pool.tile([128, NG * SV], F16, name="a16v_sb")
    a16h_sb = const_pool.tile([128, NG * SH], F16, name="a16h_sb")
    spre_sb = const_pool.tile([128, NG * SV], F16, name="spre_sb")
    ident = const_pool.tile([128, 128], F16, name="ident")
    xv_ap, xh_ap = aps["xv"], aps["xh"]
    nc.scalar.dma_start(w_p_sb[:], aps["w_p"])
    nc.sync.dma_start(xtv[0][:], xv_ap[:, :4096])
    nc.scalar.dma_start(xtv[1][:], xv_ap[:, 4096:])
    nc.scalar.dma_start(spre_sb[:], aps["s_pre"])
    nc.scalar.dma_start(a16v_sb[:], aps["a16v"])
    nc.scalar.dma_start(a16h_sb[:], aps["a16h"])
    nc.sync.dma_start(xth[0][:], xh_ap[:, :4096])
    nc.scalar.dma_start(wc[0][:], aps["w_cba"][:, :4096])
    nc.sync.dma_start(wt[0][:], aps["w_t"][:, :4096])
    nc.scalar.dma_start(xth[1][:], xh_ap[:, 4096:])
    nc.sync.dma_start(wt[1][:], aps["w_t"][:, 4096:])
    nc.scalar.dma_start(wc[1][:], aps["w_cba"][:, 4096:])
    nc.sync.dma_start(wpt_sb[:], aps["wpt"])
    make_identity(nc, ident[:])

    def xt_oct_v(o):   # per octet: own (c2, s64) = 128 cols
        return xtv[o // 32][:, (o % 32) * 128:(o % 32) * 128 + 128]

    xt_v_hi = xt_oct_v  # intra reads the same own-half columns

    def xt_oct_h(o):
        return xth[o // 32][:, (o % 32) * 128:(o % 32) * 128 + 128]

    def w_t_oct(o):
        return wt[o // 32][:, (o % 32) * 128:(o % 32) * 128 + 128]

    def w_cba_g(g):
        return wc[g // 16][:, (g % 16) * 256:(g % 16) * 256 + 256]

    def w_p_oct(o):
        return w_p_sb[:, o * 64:o * 64 + 64]

    def cp(k, dst, src):
        if k % 2 == 0:
            nc.scalar.copy(dst, src)
        else:
            nc.vector.tensor_copy(dst, src)

    # ---- phase G, both directions ----
    gall_v = g_pool.tile([128, 8 * 512], F16, name="gall_v")   # (q8,g4,c2,s64)
    sall_v = g_pool.tile([128, 8 * 512], F16, name="sall_v")   # (q8,g4,c2,s64)
    gall_h = g_pool.tile([128, 8 * 512], F16, name="gall_h")   # (q8,g4,c4,s32)
    sall_h = g_pool.tile([128, 8 * 512], F16, name="sall_h")   # (q8,g4,c4,s32)

    for q in range(8):   # vertical: 8 octets per PSUM bank
        ps_g = psG.tile([128, 512], F32, tag="ps_g")
        for k in range(8):
            o = q * 8 + k
            half = (o % 2) * 64
            col = (k // 2) * 128
            nc.tensor.matmul(ps_g[half:half + 64, col:col + 128],
                             w_p_oct(o), xt_oct_v(o),
                             start=True, stop=True, skip_group_check=True,
                             tile_position=(0, half))
        cp(q, gall_v[:, q * 512:(q + 1) * 512], ps_g[:])
    for q in range(8):   # horizontal: 8 octets per PSUM bank
        ps_g = psG.tile([128, 512], F32, tag="ps_g")
        for k in range(8):
            o = q * 8 + k
            half = (o % 2) * 64
            col = (k // 2) * 128
            nc.tensor.matmul(ps_g[half:half + 64, col:col + 128],
                             w_p_oct(o), xt_oct_h(o),
                             start=True, stop=True, skip_group_check=True,
                             tile_position=(0, half))
        cp(q + 1, gall_h[:, q * 512:(q + 1) * 512], ps_g[:])

    # ---- chunk-state recurrences (gpsimd, SBUF only) ----
    gv = gall_v[:].rearrange("p (q g c s) -> p q g c s", q=8, g=4, c=2, s=SV)
    sv = sall_v[:].rearrange("p (q g c s) -> p q g c s", q=8, g=4, c=2, s=SV)
    av = a16v_sb[:].rearrange("p (q g s) -> p q g s", q=8, g=4, s=SV)
    spre = spre_sb[:].rearrange("p (q g s) -> p q g s", q=8, g=4, s=SV)
    for i, (lo, hi) in enumerate(((0, 4), (4, 8))):
        eng = nc.vector if i == 0 else nc.gpsimd
        svq, gvq, avq = sv[:, lo:hi], gv[:, lo:hi], av[:, lo:hi]
        eng.tensor_copy(svq[:, :, :, 0], spre[:, lo:hi])
        eng.tensor_mul(svq[:, :, :, 1], spre[:, lo:hi], avq)
        eng.tensor_add(svq[:, :, :, 1], svq[:, :, :, 1], gvq[:, :, :, 0])
    gh = gall_h[:].rearrange("p (q g c s) -> p q g c s", q=8, g=4, c=4, s=SH)
    sh = sall_h[:].rearrange("p (q g c s) -> p q g c s", q=8, g=4, c=4, s=SH)
    ah = a16h_sb[:].rearrange("p (q g s) -> p q g s", q=8, g=4, s=SH)
    nc.gpsimd.memset(sh[:, :, :, 0], 0.0)
    for i, (lo, hi) in enumerate(((0, 4), (4, 8))):
        eng = nc.vector if i == 0 else nc.gpsimd
        shq, ghq, ahq = sh[:, lo:hi], gh[:, lo:hi], ah[:, lo:hi]
        nc.scalar.copy(shq[:, :, :, 1], ghq[:, :, :, 0])
        eng.tensor_mul(shq[:, :, :, 2], shq[:, :, :, 1], ahq)
        eng.tensor_add(shq[:, :, :, 2], shq[:, :, :, 2], ghq[:, :, :, 1])
        eng.tensor_mul(shq[:, :, :, 3], shq[:, :, :, 2], ahq)
        eng.tensor_add(shq[:, :, :, 3], shq[:, :, :, 3], ghq[:, :, :, 2])

    # ---- phase B, both directions ----
    y_v = y_pool.tile([128, NOCT * 128], F16, tag="y", name="y_v")
    y_h = y_pool.tile([128, NOCT * 128], F16, tag="y", name="y_h")
    for og in range(16):  # vertical, own chunks {2,3} only
        ps_yw = psyw.tile([128, 512], F32, tag="ps_yw")
        for oo in range(4):
            o = og * 4 + oo
            nc.tensor.matmul(ps_yw[:, oo * 128:oo * 128 + 128],
                             xt_v_hi(o), w_t_oct(o),
                             start=(oo == 0), stop=False, skip_group_check=True)
        for gg in range(2):
            g = og * 2 + gg
            nc.tensor.matmul(ps_yw[:, gg * 256:gg * 256 + 256],
                             sall_v[:, g * 128:g * 128 + 128], w_cba_g(g),
                             start=False, stop=(gg == 1), skip_group_check=True)
        y_dst = y_v[:].rearrange("p (i og o e) -> p i og o e",
                                 i=Q, og=16, o=4, e=8)[:, :, og]
        ps_src = ps_yw[:].rearrange("p (o i e) -> p i o e", o=4, i=Q, e=8)
        cp(og, y_dst, ps_src)
    for og in range(16):  # horizontal, all 4 chunks
        ps_yw = psyw.tile([128, 512], F32, tag="ps_yw")
        for oo in range(4):
            o = og * 4 + oo
            nc.tensor.matmul(ps_yw[:, oo * 128:oo * 128 + 128], xt_oct_h(o),
                             w_t_oct(o),
                             start=(oo == 0), stop=False, skip_group_check=True)
        for gg in range(2):
            g = og * 2 + gg
            nc.tensor.matmul(ps_yw[:, gg * 256:gg * 256 + 256],
                             sall_h[:, g * 128:g * 128 + 128], w_cba_g(g),
                             start=False, stop=(gg == 1), skip_group_check=True)
        y_dst = y_h[:].rearrange("p (i og o e) -> p i og o e",
                                 i=Q, og=16, o=4, e=8)[:, :, og]
        ps_src = ps_yw[:].rearrange("p (o i e) -> p i o e", o=4, i=Q, e=8)
        cp(og + 1, y_dst, ps_src)

    # ---- transpose to [d, t] and merge; t = iw*128 + cv*64 + s',
    # where the host packs vertical seqs in order s' = ih*4 + ch  ----
    yt = yt_pool.tile([128, 4 * 2048], F16, name="yt")  # (dc4, t2048)
    # vertical: y_v partitions (cv2, s'64) == t order within block iw
    for iw in range(16):
        ps_yt = psA.tile([128, 512], F16, tag="ps_t")
        for dc in range(4):
            nc.tensor.transpose(
                ps_yt[:, dc * 128:(dc + 1) * 128],
                y_v[:, iw * 512 + dc * 128:iw * 512 + (dc + 1) * 128], ident)
        # contiguous: yt[:, (dc, iw-block)] <- ps_yt[:, (dc, m)]
        dst = yt[:].rearrange("p (dc b m) -> p b dc m", dc=4, b=16)[:, iw]
        cp(iw, dst, ps_yt[:].rearrange("p (dc m) -> p dc m", dc=4))
    # horizontal: y_h partitions (ch4, w32); accumulate with run-4 writes
    for ih in range(16):
        ps_yt = psA.tile([128, 512], F16, tag="ps_t")
        for dc in range(4):
            nc.tensor.transpose(
                ps_yt[:, dc * 128:(dc + 1) * 128],
                y_h[:, ih * 512 + dc * 128:ih * 512 + (dc + 1) * 128], ident)
        # t = iw*128 + cv*64 + ih*4 + ch ; ps_yt cols (dc4, ch4, cv2, iw16)
        for cv in range(2):
            dst = yt[:].rearrange("p (dc iw cv u) -> p cv dc iw u",
                                  dc=4, iw=16, cv=2, u=64)[
                :, cv, :, :, ih * 4:ih * 4 + 4]
            srcv = ps_yt[:].rearrange("p (dc ch cv iw) -> p cv dc iw ch",
                                      dc=4, ch=4, cv=2, iw=16)[:, cv]
            nc.vector.tensor_add(dst, dst, srcv)

    # ---- single fused projection over 2048 tokens ----
    for blk in range(16):
        ps_o = psout.tile([128, 512], F32, tag="ps_o")
        for dc in range(4):
            nc.tensor.matmul(ps_o[:],
                             yt[:, dc * 2048 + blk * 128:dc * 2048 + blk * 128 + 128],
                             wpt_sb[:, dc * 512:dc * 512 + 512],
                             start=(dc == 0), stop=(dc == 3))
        out_sb = out_pool.tile([128, 512], F16, tag="osb")
        cp(blk, out_sb[:], ps_o[:])
        nc.sync.dma_start(aps["z"][blk * 128:(blk + 1) * 128, :], out_sb[:])


def build_program(n_cores=8):
    nc = bacc.Bacc("TRN2", target_bir_lowering=False, debug=False,
                   enable_asserts=False, num_devices=n_cores)
    aps = {
        "xv": nc.dram_tensor("xv", [128, NOCT * 128], F16, kind="ExternalInput").ap(),
        "xh": nc.dram_tensor("xh", [128, NOCT * 128], F16, kind="ExternalInput").ap(),
        "w_t": nc.dram_tensor("w_t", [128, NOCT * 128], F16, kind="ExternalInput").ap(),
        "w_p": nc.dram_tensor("w_p", [128, NOCT * 64], F16, kind="ExternalInput").ap(),
        "w_cba": nc.dram_tensor("w_cba", [128, NG * 256], F16,
                                kind="ExternalInput").ap(),
        "wpt": nc.dram_tensor("wpt", [128, 4 * 512], F16, kind="ExternalInput").ap(),
        "a16v": nc.dram_tensor("a16v", [128, NG * SV], F16, kind="ExternalInput").ap(),
        "a16h": nc.dram_tensor("a16h", [128, NG * SH], F16, kind="ExternalInput").ap(),
        "s_pre": nc.dram_tensor("s_pre", [128, NG * SV], F16,
                                kind="ExternalInput").ap(),
        "z": nc.dram_tensor("z", [2048, D], F16, kind="ExternalOutput").ap(),
    }
    with tile.TileContext(nc) as tc:
        with ExitStack() as ctx:
            _kernel_body(ctx, tc, aps)
    nc.compile()
    return nc


_PROGRAM = None


def _get_program():
    global _PROGRAM
    if _PROGRAM is None:
        _PROGRAM = build_program()
    return _PROGRAM


def make_in_maps(x, A, Bm, Cm, D_skip, Wp):
    w_t, w_p, w_cba, wpt, a16v, a16h = _precompute_weights(A, Bm, Cm, D_skip, Wp)
    xg = np.ascontiguousarray(x, dtype=np.float32).reshape(B, H, W, D)
    A64 = A.astype(np.float64)
    # B-normalized boundary state after w=31 (B is folded into W_CBA)
    P = A64[None] ** np.arange(31, -1, -1, dtype=np.float64)[:, None, None]
    s_bnd = np.einsum("udn,bsud->bsdn", P,
                      xg[:, VPERM, :32, :].astype(np.float64))
    # pack [b, s(h), d, n] -> [(o2,d8,n)=128, (q8, g4, s64)]
    sp = s_bnd.reshape(B, SV, NG, 2, 8, N)       # b, s, g32, o2, d8, n
    sp = sp.transpose(0, 3, 4, 5, 2, 1)          # b, o2, d8, n, g, s
    sp = sp.reshape(B, 128, NG * SV).astype(NP_CDT)
    in_maps = []
    for k in range(8):
        b, half = k // 2, k % 2
        # vertical: 64 h-row seqs in order s' = (h%16)*4 + h//16
        xs_v = np.ascontiguousarray(xg[b, VPERM, 32 * half:32 * half + 32])
        # horizontal: 32 own w-col seqs; positions = h 0..63
        xs_h = np.ascontiguousarray(
            xg[b, :, 32 * half:32 * half + 32].transpose(1, 0, 2))
        s_pre = sp[b] if half == 1 else np.zeros_like(sp[b])
        # (sp already packed with the permuted seq order)
        in_maps.append({
            "xv": _pack_xt(xs_v, SV, 2), "xh": _pack_xt(xs_h, SH, 4),
            "w_t": w_t, "w_p": w_p, "w_cba": w_cba, "wpt": wpt,
            "a16v": a16v, "a16h": a16h, "s_pre": s_pre,
        })
    return in_maps


def assemble_output(results, b_proj):
    out = np.zeros((B, H, W, D), np.float32)
    for k in range(8):
        b, half = k // 2, k % 2
        z = results[k]["z"].astype(np.float32)      # [2048,512], t=(iw,cv,ih,ch)
        zr = z.reshape(16, 2, 16, 4, D).transpose(3, 2, 1, 0, 4)  # ch,ih,cv,iw
        out[b, :, 32 * half:32 * half + 32] = zr.reshape(H, SH, D)
    out += np.asarray(b_proj, dtype=np.float32)
    return out.reshape(B, H * W, D)


def kernel(x, h, w, A, Bm, Cm, D_skip, Wp, b_proj, **_kw):
    nc = _get_program()
    in_maps = make_in_maps(np.asarray(x), np.asarray(A), np.asarray(Bm),
                           np.asarray(Cm), np.asarray(D_skip), np.asarray(Wp))
    res = run_bass_kernel_spmd(nc, in_maps, list(range(8)))
    return assemble_output(res.results, np.asarray(b_proj))
